# revision 9
# baseline (speedup 1.0000x reference)
"""FeaStConv dual-branch GNN message passing on 8 Trainium2 NeuronCores.

Sharding: branch v on cores 0-3, branch f on cores 4-7; each core owns a
12500-node destination range. Host reorders edges by destination block
(64 nodes), pre-gathers transposed source/dest features (bf16), device does
all float math: per-tile matmuls for x@W / (x_s-x_d)@U, softmax on-device,
one-hot scatter matmuls accumulating per-block in PSUM.
"""
import sys, types
import numpy as np

sys.path.insert(0, '/opt/trn_rl_repo')

N = 50000
IN_CH = 64
HEADS = 4
OUT_CH = 32
P = 128
NPC = 12500           # nodes per core
BLK = 64              # dst nodes per block
NBLK = 196            # blocks per core (196*64 = 12544)
NPAD = NBLK * BLK
CH = 8               # tiles per chunk
SCT = 48              # tiles per superchunk
NCORES = 8


def _register_ntff_hook():
    import antenv
    if "antenv.axon_hooks" in sys.modules:
        return
    mod = types.ModuleType("antenv.axon_hooks")
    _h = [None]
    mod.set_axon_ntff_profile_hook = lambda h: _h.__setitem__(0, h)
    mod.get_axon_ntff_profile_hook = lambda: _h[0]
    sys.modules["antenv.axon_hooks"] = mod
    antenv.axon_hooks = mod
    if "/root/.axon_site" not in sys.path:
        sys.path.insert(0, "/root/.axon_site")
    try:
        from trn_agent_boot.trn_boot import _ntff_profile_via_ctypes
        mod.set_axon_ntff_profile_hook(_ntff_profile_via_ctypes('/opt/axon/libaxon_pjrt.so'))
    except Exception:
        pass


def _prep_core(x16, src, dst, lo):
    """Per-core edge layout. Returns dict with per-block counts and sorted
    (global-src, global-dst, local-slot) arrays."""
    sel = (dst >= lo) & (dst < lo + NPC)
    s = src[sel]
    d = (dst[sel] - lo).astype(np.int64)
    order = np.argsort(d, kind='stable')
    s = s[order]
    d = d[order]
    blk = d >> 6
    cnt = np.bincount(blk, minlength=NBLK).astype(np.int64)
    deg = np.bincount(d, minlength=NPAD).astype(np.float32)
    return {"s": s, "d": d, "cnt": cnt, "deg": deg}


def _build_core_arrays(ml, core, TPB, base, NT):
    import ml_dtypes
    x16, W, U, c, b = core["x16"], core["W"], core["U"], core["c"], core["b"]
    s, d, cnt = core["g"]["s"], core["g"]["d"], core["g"]["cnt"]
    E_pad = NT * P
    srcg = np.zeros(E_pad, np.int64)
    dstg = np.zeros(E_pad, np.int64)
    dl = np.full(E_pad, -1.0, np.float32)
    # place each block's edges at its tile base
    cstart = np.concatenate([[0], np.cumsum(cnt)])
    for k in range(NBLK):
        n_k = int(cnt[k])
        if n_k == 0:
            continue
        p0 = base[k] * P
        srcg[p0:p0 + n_k] = s[cstart[k]:cstart[k] + n_k]
        dstg[p0:p0 + n_k] = d[cstart[k]:cstart[k] + n_k] + core["lo"]
        dl[p0:p0 + n_k] = (d[cstart[k]:cstart[k] + n_k] - BLK * k).astype(np.float32)
    xsd = np.empty((P, E_pad), ml_dtypes.bfloat16)
    xsd[:IN_CH, :] = x16[srcg].T
    xsd[IN_CH:, :] = x16[dstg].T
    dl16 = np.ascontiguousarray(dl.reshape(NT, P).T).astype(ml_dtypes.bfloat16)
    # plain layout [h*32+ch]; bottom 64 rows zero (x_dst doesn't enter xjw)
    Wcm = np.zeros((P, P), np.float32)
    Wcm[:IN_CH] = W
    UUc = np.concatenate([U, -U], axis=0)  # [128, 4]
    degp = np.ascontiguousarray(core["g"]["deg"].reshape(NBLK // 2, P).T)  # [128, 98]
    return {
        "xsd": xsd,
        "dl": dl16,
        "wcm": Wcm.astype(ml_dtypes.bfloat16),
        "uuc": UUc.astype(ml_dtypes.bfloat16),
        "crep": np.tile(c[None, :], (P, 1)).astype(np.float32),
        "brep": np.tile(b[None, :], (P, 1)).astype(np.float32),
        "degp": degp.astype(np.float32),
        "iota": np.tile(np.arange(BLK, dtype=np.float32)[None, :], (P, CH)).astype(ml_dtypes.bfloat16),
    }


def _build_program(TPB, NT):
    import concourse.bass as bass
    import concourse.mybir as mybir
    import concourse.bacc as bacc
    from concourse.tile import TileContext

    dt = mybir.dt
    NSC = NT // SCT
    NCH = NT // CH
    # block index / first / last flags per tile
    blk_of = np.repeat(np.arange(NBLK), TPB)
    t0 = np.concatenate([[0], np.cumsum(TPB)])

    nc = bacc.Bacc("TRN2", target_bir_lowering=False, debug=False, num_devices=NCORES)
    xsd_d = nc.dram_tensor("xsd", [P, NT * P], dt.bfloat16, kind="ExternalInput").ap()
    dl_d = nc.dram_tensor("dl", [P, NT], dt.bfloat16, kind="ExternalInput").ap()
    wcm_d = nc.dram_tensor("wcm", [P, P], dt.bfloat16, kind="ExternalInput").ap()
    uuc_d = nc.dram_tensor("uuc", [P, 4], dt.bfloat16, kind="ExternalInput").ap()
    crep_d = nc.dram_tensor("crep", [P, 4], dt.float32, kind="ExternalInput").ap()
    brep_d = nc.dram_tensor("brep", [P, OUT_CH], dt.float32, kind="ExternalInput").ap()
    degp_d = nc.dram_tensor("degp", [P, NBLK // 2], dt.float32, kind="ExternalInput").ap()
    iota_d = nc.dram_tensor("iota", [P, BLK * CH], dt.bfloat16, kind="ExternalInput").ap()
    out_d = nc.dram_tensor("out", [NPAD, OUT_CH], dt.float32, kind="ExternalOutput").ap()

    def APn(t, dims, off=0):
        a = t[:]
        return bass.AP(a.tensor, a.offset + off, [a.ap[0]] + dims)

    with TileContext(nc) as tc:
        with tc.tile_pool(name="const", bufs=1) as cp, \
             tc.tile_pool(name="mega", bufs=3) as mp, \
             tc.tile_pool(name="work", bufs=6) as wp, \
             tc.tile_pool(name="qp", bufs=2) as qp, \
             tc.tile_pool(name="fin", bufs=1) as fp, \
             tc.tile_pool(name="psA", bufs=2, space="PSUM") as psA, \
             tc.tile_pool(name="psU", bufs=2, space="PSUM") as psU, \
             tc.tile_pool(name="psG", bufs=2, space="PSUM") as psG:

            wcm = cp.tile([P, P], dt.bfloat16)
            uuc = cp.tile([P, 4], dt.bfloat16)
            crep = cp.tile([P, 4], dt.float32)
            brep = cp.tile([P, OUT_CH], dt.float32)
            degp = cp.tile([P, NBLK // 2], dt.float32)
            iota = cp.tile([P, BLK * CH], dt.bfloat16)
            dlb = cp.tile([P, NT], dt.bfloat16)
            expc = cp.tile([P, 4], dt.float32)
            nc.sync.dma_start(out=wcm[:], in_=wcm_d[:])
            nc.sync.dma_start(out=uuc[:], in_=uuc_d[:])
            nc.sync.dma_start(out=crep[:], in_=crep_d[:])
            nc.sync.dma_start(out=brep[:], in_=brep_d[:])
            nc.sync.dma_start(out=degp[:], in_=degp_d[:])
            nc.sync.dma_start(out=iota[:], in_=iota_d[:])
            nc.sync.dma_start(out=dlb[:], in_=dl_d[:])
            nc.scalar.activation(expc[:], crep[:], mybir.ActivationFunctionType.Exp)

            fin = fp.tile([P, (NBLK // 2) * P], dt.float32)

            acc = None
            for sc in range(NSC):
                xm = mp.tile([P, SCT * P], dt.bfloat16, tag="xm", name="xm")
                nc.sync.dma_start(out=xm[:], in_=xsd_d[:, sc * SCT * P:(sc + 1) * SCT * P])
                pU = psU.tile([P, SCT * 4], dt.float32, tag="pU", name="pU")
                qe = qp.tile([P, SCT * 4], dt.float32, tag="qe", name="qe")
                qb = qp.tile([P, SCT * 4], dt.float32, tag="qb", name="qb")
                den = qp.tile([P, SCT], dt.float32, tag="den", name="den")
                rec = qp.tile([P, SCT], dt.float32, tag="rec", name="rec")

                chunk_data = []
                for ci in range(SCT // CH):
                    pA = psA.tile([P, CH * P], dt.float32, tag="pA", name="pA")
                    for i in range(CH):
                        t = sc * SCT + ci * CH + i
                        lhs = xm[:, (ci * CH + i) * P:(ci * CH + i + 1) * P]
                        nc.tensor.matmul(out=pA[:, i * P:(i + 1) * P], lhsT=lhs,
                                         rhs=wcm[:], start=True, stop=True)
                        nc.tensor.matmul(out=pU[:, (ci * CH + i) * 4:(ci * CH + i + 1) * 4],
                                         lhsT=lhs, rhs=uuc[:], start=True, stop=True)
                    chunk_data.append(pA)

                # softmax over the whole superchunk
                nc.scalar.activation(qe[:], pU[:], mybir.ActivationFunctionType.Exp)
                nc.vector.tensor_tensor(
                    out=APn(qb, [[4, SCT], [1, 4]]),
                    in0=APn(qe, [[4, SCT], [1, 4]]),
                    in1=APn(expc, [[0, SCT], [1, 4]]),
                    op=mybir.AluOpType.mult)
                nc.vector.tensor_reduce(
                    out=den[:], in_=APn(qb, [[4, SCT], [1, 4]]),
                    op=mybir.AluOpType.add, axis=mybir.AxisListType.X)
                nc.vector.reciprocal(out=rec[:], in_=den[:])
                nc.vector.tensor_tensor(
                    out=APn(qe, [[4, SCT], [1, 4]]),
                    in0=APn(qb, [[4, SCT], [1, 4]]),
                    in1=APn(rec, [[1, SCT], [0, 4]]),
                    op=mybir.AluOpType.mult)

                for ci in range(SCT // CH):
                    pA = chunk_data[ci]
                    stg = wp.tile([P, CH * P], dt.bfloat16, tag="stg", name="stg")
                    oh = wp.tile([P, CH * BLK], dt.bfloat16, tag="oh", name="oh")
                    # z~ = q * xjw  (plain layout: col i*128 + h*32 + ch)
                    nc.vector.tensor_tensor(
                        out=APn(stg, [[P, CH], [32, 4], [1, 32]]),
                        in0=APn(pA, [[P, CH], [32, 4], [1, 32]]),
                        in1=APn(qe, [[4, CH], [1, 4], [0, 32]], off=ci * CH * 4),
                        op=mybir.AluOpType.mult)
                    # onehot[e, i*64+dd] = (iota == dl)
                    nc.vector.tensor_tensor(
                        out=APn(oh, [[1, CH * BLK]]),
                        in0=APn(iota, [[1, CH * BLK]]),
                        in1=APn(dlb, [[1, CH], [0, BLK]], off=sc * SCT + ci * CH),
                        op=mybir.AluOpType.is_equal)
                    for i in range(CH):
                        t = sc * SCT + ci * CH + i
                        k = int(blk_of[t])
                        if k % 2 == 0 and t == t0[k]:
                            acc = psG.tile([P, P], dt.float32, tag="acc", name="acc")
                        half = (k % 2) * BLK
                        nc.tensor.matmul(
                            out=acc[half:half + BLK, :],
                            lhsT=oh[:, i * BLK:(i + 1) * BLK],
                            rhs=stg[:, i * P:(i + 1) * P],
                            start=(t == t0[k]), stop=(t == t0[k + 1] - 1))
                        if k % 2 == 1 and t == t0[k + 1] - 1:
                            m = k // 2
                            nc.scalar.copy(out=fin[:, m * P:(m + 1) * P], in_=acc[:])

            # finale: h-sum, deg-divide, bias, leaky relu
            NH = NBLK // 2
            hs = fp.tile([P, NH * OUT_CH], dt.float32)
            nc.vector.tensor_reduce(
                out=APn(hs, [[32, NH], [1, 32]]),
                in_=APn(fin, [[P, NH], [1, 32], [32, 4]]),
                op=mybir.AluOpType.add, axis=mybir.AxisListType.X)
            dmx = fp.tile([P, NH], dt.float32)
            nc.vector.tensor_scalar(out=dmx[:], in0=degp[:], scalar1=1.0, scalar2=None,
                                    op0=mybir.AluOpType.max)
            drc = fp.tile([P, NH], dt.float32)
            nc.vector.reciprocal(out=drc[:], in_=dmx[:])
            o1 = fp.tile([P, NH * OUT_CH], dt.float32)
            nc.vector.tensor_tensor(
                out=APn(o1, [[32, NH], [1, 32]]),
                in0=APn(hs, [[32, NH], [1, 32]]),
                in1=APn(drc, [[1, NH], [0, 32]]),
                op=mybir.AluOpType.mult)
            nc.vector.tensor_tensor(
                out=APn(o1, [[32, NH], [1, 32]]),
                in0=APn(o1, [[32, NH], [1, 32]]),
                in1=APn(brep, [[0, NH], [1, 32]]),
                op=mybir.AluOpType.add)
            o2 = fp.tile([P, NH * OUT_CH], dt.float32)
            nc.vector.tensor_scalar(out=o2[:], in0=o1[:], scalar1=0.2, scalar2=None,
                                    op0=mybir.AluOpType.mult)
            nc.vector.tensor_tensor(out=o1[:], in0=o1[:], in1=o2[:],
                                    op=mybir.AluOpType.max)
            # out[128j + p, ch] = o1[p, j*32+ch]
            out_ap = bass.AP(out_d.tensor, out_d.offset,
                             [[OUT_CH, P], [P * OUT_CH, NH], [1, OUT_CH]])
            nc.sync.dma_start(out=out_ap, in_=APn(o1, [[32, NH], [1, 32]]))
    nc.compile()
    return nc


def kernel(x_v, edge_index_v, x_f, edge_index_f, Wv, Uv, cv, bv, Wf, Uf, cf, bf):
    _register_ntff_hook()
    import ml_dtypes
    from concourse import bass_utils

    x_v = np.asarray(x_v, np.float32)
    x_f = np.asarray(x_f, np.float32)
    cores = []
    for bi, (x, ei, W, U, c, b) in enumerate([
            (x_v, edge_index_v, Wv, Uv, cv, bv),
            (x_f, edge_index_f, Wf, Uf, cf, bf)]):
        ei = np.asarray(ei)
        s0, d0 = ei[0].astype(np.int64), ei[1].astype(np.int64)
        m = s0 != d0
        loops = np.arange(N, dtype=np.int64)
        src = np.concatenate([s0[m], loops])
        dst = np.concatenate([d0[m], loops])
        x16 = x.astype(ml_dtypes.bfloat16)
        for j in range(4):
            lo = j * NPC
            cores.append({
                "x16": x16, "W": np.asarray(W, np.float32),
                "U": np.asarray(U, np.float32), "c": np.asarray(c, np.float32),
                "b": np.asarray(b, np.float32), "lo": lo,
                "g": _prep_core(x16, src, dst, lo),
            })

    tn = np.stack([np.ceil(c["g"]["cnt"] / P).astype(np.int64) for c in cores])
    TPB = tn.max(axis=0)
    TPB = np.maximum(TPB, 1)
    NT = int(TPB.sum())
    pad = (-NT) % SCT
    TPB[NBLK - 1] += pad
    NT += pad
    base = np.concatenate([[0], np.cumsum(TPB)])[:-1]

    in_maps = []
    for c in cores:
        arrs = _build_core_arrays(None, c, TPB, base, NT)
        in_maps.append(arrs)

    nc = _build_program(TPB, NT)
    res = bass_utils.run_bass_kernel_spmd(
        nc, in_maps, core_ids=list(range(NCORES)),
        trace=bool(int(__import__("os").environ.get("KERNEL_TRACE", "0"))))
    kernel.last_result = res
    out_v = np.concatenate([res.results[j]["out"][:NPC] for j in range(4)])
    out_f = np.concatenate([res.results[4 + j]["out"][:NPC] for j in range(4)])
    return out_v, out_f


# revision 17
# speedup vs baseline: 1.1679x; 1.1679x over previous
"""FeaStConv dual-branch GNN message passing on 8 Trainium2 NeuronCores.

Sharding: branch v on cores 0-3, branch f on cores 4-7; each core owns a
12500-node destination range. Host reorders edges by destination block
(64 nodes), pre-gathers transposed source/dest features (bf16), device does
all float math: per-tile matmuls for x@W / (x_s-x_d)@U, softmax on-device,
one-hot scatter matmuls accumulating per-block in PSUM.
"""
import sys, types
import numpy as np

sys.path.insert(0, '/opt/trn_rl_repo')

N = 50000
IN_CH = 64
HEADS = 4
OUT_CH = 32
P = 128
NPC = 12500           # nodes per core
BLK = 64              # dst nodes per block
NBLK = 196            # blocks per core (196*64 = 12544)
NPAD = NBLK * BLK
CH = 8               # tiles per chunk
SCT = 24              # tiles per superchunk
NCORES = 8


def _register_ntff_hook():
    import antenv
    if "antenv.axon_hooks" in sys.modules:
        return
    mod = types.ModuleType("antenv.axon_hooks")
    _h = [None]
    mod.set_axon_ntff_profile_hook = lambda h: _h.__setitem__(0, h)
    mod.get_axon_ntff_profile_hook = lambda: _h[0]
    sys.modules["antenv.axon_hooks"] = mod
    antenv.axon_hooks = mod
    if "/root/.axon_site" not in sys.path:
        sys.path.insert(0, "/root/.axon_site")
    try:
        from trn_agent_boot.trn_boot import _ntff_profile_via_ctypes
        mod.set_axon_ntff_profile_hook(_ntff_profile_via_ctypes('/opt/axon/libaxon_pjrt.so'))
    except Exception:
        pass


def _prep_core(x16, src, dst, lo):
    """Per-core edge layout. Returns dict with per-block counts and sorted
    (global-src, global-dst, local-slot) arrays."""
    sel = (dst >= lo) & (dst < lo + NPC)
    s = src[sel]
    d = (dst[sel] - lo).astype(np.int64)
    order = np.argsort(d, kind='stable')
    s = s[order]
    d = d[order]
    blk = d >> 6
    cnt = np.bincount(blk, minlength=NBLK).astype(np.int64)
    deg = np.bincount(d, minlength=NPAD).astype(np.float32)
    return {"s": s, "d": d, "cnt": cnt, "deg": deg}


def _build_core_arrays(ml, core, TPB, base, NT):
    import ml_dtypes
    x16, W, U, c, b = core["x16"], core["W"], core["U"], core["c"], core["b"]
    s, d, cnt = core["g"]["s"], core["g"]["d"], core["g"]["cnt"]
    E_pad = NT * P
    srcg = np.zeros(E_pad, np.int64)
    dstg = np.zeros(E_pad, np.int64)
    dl = np.full(E_pad, -1.0, np.float32)
    # place each block's edges at its tile base
    cstart = np.concatenate([[0], np.cumsum(cnt)])
    for k in range(NBLK):
        n_k = int(cnt[k])
        if n_k == 0:
            continue
        p0 = base[k] * P
        srcg[p0:p0 + n_k] = s[cstart[k]:cstart[k] + n_k]
        dstg[p0:p0 + n_k] = d[cstart[k]:cstart[k] + n_k] + core["lo"]
        dl[p0:p0 + n_k] = (d[cstart[k]:cstart[k] + n_k] - BLK * k).astype(np.float32)
    xsd = np.empty((P, E_pad), ml_dtypes.bfloat16)
    xsd[:IN_CH, :] = x16[srcg].T
    xsd[IN_CH:, :] = x16[dstg].T
    dl16 = np.ascontiguousarray(dl.reshape(NT, P).T).astype(ml_dtypes.bfloat16)
    # plain layout [h*32+ch]; bottom 64 rows zero (x_dst doesn't enter xjw)
    Wcm = np.zeros((P, P), np.float32)
    Wcm[:IN_CH] = W
    UUc = np.concatenate([U, -U], axis=0)  # [128, 4]
    degp = np.ascontiguousarray(core["g"]["deg"].reshape(NBLK // 2, P).T)  # [128, 98]
    return {
        "xsd": xsd,
        "dl": dl16,
        "wcm": Wcm.astype(ml_dtypes.bfloat16),
        "uuc": UUc.astype(ml_dtypes.bfloat16),
        "crep": np.tile(c[None, :], (P, 1)).astype(np.float32),
        "cvec": np.tile(c, SCT)[None, :].astype(ml_dtypes.bfloat16),
        "brep": np.tile(b[None, :], (P, 1)).astype(np.float32),
        "degp": degp.astype(np.float32),
        "iota": np.tile(np.arange(BLK, dtype=np.float32)[None, :], (P, CH)).astype(ml_dtypes.bfloat16),
    }


def _build_program(TPB, NT):
    import concourse.bass as bass
    import concourse.mybir as mybir
    import concourse.bacc as bacc
    from concourse.tile import TileContext

    dt = mybir.dt
    NSC = NT // SCT
    NCH = NT // CH
    # block index / first / last flags per tile
    blk_of = np.repeat(np.arange(NBLK), TPB)
    t0 = np.concatenate([[0], np.cumsum(TPB)])

    nc = bacc.Bacc("TRN2", target_bir_lowering=False, debug=False, num_devices=NCORES)
    xsd_d = nc.dram_tensor("xsd", [P, NT * P], dt.bfloat16, kind="ExternalInput").ap()
    dl_d = nc.dram_tensor("dl", [P, NT], dt.bfloat16, kind="ExternalInput").ap()
    wcm_d = nc.dram_tensor("wcm", [P, P], dt.bfloat16, kind="ExternalInput").ap()
    uuc_d = nc.dram_tensor("uuc", [P, 4], dt.bfloat16, kind="ExternalInput").ap()
    crep_d = nc.dram_tensor("crep", [P, 4], dt.float32, kind="ExternalInput").ap()
    cvec_d = nc.dram_tensor("cvec", [1, SCT * 4], dt.bfloat16, kind="ExternalInput").ap()
    brep_d = nc.dram_tensor("brep", [P, OUT_CH], dt.float32, kind="ExternalInput").ap()
    degp_d = nc.dram_tensor("degp", [P, NBLK // 2], dt.float32, kind="ExternalInput").ap()
    iota_d = nc.dram_tensor("iota", [P, BLK * CH], dt.bfloat16, kind="ExternalInput").ap()
    out_d = nc.dram_tensor("out", [NPAD, OUT_CH], dt.float32, kind="ExternalOutput").ap()

    def APn(t, dims, off=0):
        a = t[:]
        return bass.AP(a.tensor, a.offset + off, [a.ap[0]] + dims)

    with TileContext(nc) as tc:
        with tc.tile_pool(name="const", bufs=1) as cp, \
             tc.tile_pool(name="mega", bufs=3) as mp, \
             tc.tile_pool(name="work", bufs=6) as wp, \
             tc.tile_pool(name="qp", bufs=2) as qp, \
             tc.tile_pool(name="fin", bufs=2) as fp, \
             tc.tile_pool(name="finacc", bufs=1) as fap, \
             tc.tile_pool(name="psA", bufs=2, space="PSUM") as psA, \
             tc.tile_pool(name="psU", bufs=2, space="PSUM") as psU, \
             tc.tile_pool(name="psG", bufs=2, space="PSUM") as psG:

            wcm = cp.tile([P, P], dt.bfloat16)
            uuc = cp.tile([P, 4], dt.bfloat16)
            crep = cp.tile([P, 4], dt.float32)
            brep = cp.tile([P, OUT_CH], dt.float32)
            degp = cp.tile([P, NBLK // 2], dt.float32)
            iota = cp.tile([P, BLK * CH], dt.bfloat16)
            dlb = cp.tile([P, NT], dt.bfloat16)
            cvec = cp.tile([1, SCT * 4], dt.bfloat16)
            ones1 = cp.tile([1, P], dt.bfloat16)
            expc = cp.tile([P, 4], dt.float32)
            nc.sync.dma_start(out=wcm[:], in_=wcm_d[:])
            nc.sync.dma_start(out=uuc[:], in_=uuc_d[:])
            nc.sync.dma_start(out=crep[:], in_=crep_d[:])
            nc.sync.dma_start(out=brep[:], in_=brep_d[:])
            nc.sync.dma_start(out=degp[:], in_=degp_d[:])
            nc.sync.dma_start(out=iota[:], in_=iota_d[:])
            nc.sync.dma_start(out=dlb[:], in_=dl_d[:])
            nc.sync.dma_start(out=cvec[:], in_=cvec_d[:])
            nc.vector.memset(ones1[:], 1.0)
            nc.scalar.activation(expc[:], crep[:], mybir.ActivationFunctionType.Exp)

            fin = fap.tile([P, (NBLK // 2) * P], dt.float32)

            NH = NBLK // 2
            FIN_BOUNDS = [25, 50, 75, NH]

            def emit_finale(g0, g1):
                ng = g1 - g0
                hs = fp.tile([P, ng * OUT_CH], dt.float32, tag="hs", name="hs")
                nc.vector.tensor_reduce(
                    out=APn(hs, [[32, ng], [1, 32]]),
                    in_=APn(fin, [[P, ng], [1, 32], [32, 4]], off=g0 * P),
                    op=mybir.AluOpType.add, axis=mybir.AxisListType.X)
                dmx = fp.tile([P, ng], dt.float32, tag="dmx", name="dmx")
                nc.vector.tensor_scalar(out=dmx[:], in0=degp[:, g0:g1],
                                        scalar1=1.0, scalar2=None,
                                        op0=mybir.AluOpType.max)
                drc = fp.tile([P, ng], dt.float32, tag="drc", name="drc")
                nc.vector.reciprocal(out=drc[:], in_=dmx[:])
                o1 = fp.tile([P, ng * OUT_CH], dt.float32, tag="o1", name="o1")
                nc.vector.tensor_tensor(
                    out=APn(o1, [[32, ng], [1, 32]]),
                    in0=APn(hs, [[32, ng], [1, 32]]),
                    in1=APn(drc, [[1, ng], [0, 32]]),
                    op=mybir.AluOpType.mult)
                nc.vector.tensor_tensor(
                    out=APn(o1, [[32, ng], [1, 32]]),
                    in0=APn(o1, [[32, ng], [1, 32]]),
                    in1=APn(brep, [[0, ng], [1, 32]]),
                    op=mybir.AluOpType.add)
                o2 = fp.tile([P, ng * OUT_CH], dt.float32, tag="o2", name="o2")
                nc.vector.tensor_scalar(out=o2[:], in0=o1[:], scalar1=0.2,
                                        scalar2=None, op0=mybir.AluOpType.mult)
                nc.vector.tensor_tensor(out=o1[:], in0=o1[:], in1=o2[:],
                                        op=mybir.AluOpType.max)
                out_ap = bass.AP(out_d.tensor, out_d.offset + g0 * P * OUT_CH,
                                 [[OUT_CH, P], [P * OUT_CH, ng], [1, OUT_CH]])
                nc.sync.dma_start(out=out_ap, in_=APn(o1, [[32, ng], [1, 32]]))

            acc = None
            for sc in range(NSC):
                xm = mp.tile([P, SCT * P], dt.bfloat16, tag="xm", name="xm")
                nc.sync.dma_start(out=xm[:], in_=xsd_d[:, sc * SCT * P:(sc + 1) * SCT * P])
                pU = psU.tile([P, SCT * 4], dt.float32, tag="pU", name="pU")
                nc.tensor.matmul(out=pU[:], lhsT=ones1[:], rhs=cvec[:],
                                 start=True, stop=False)
                qe = qp.tile([P, SCT * 4], dt.float32, tag="qe", name="qe")
                qb = qp.tile([P, SCT * 4], dt.float32, tag="qb", name="qb")
                den = qp.tile([P, SCT], dt.float32, tag="den", name="den")
                rec = qp.tile([P, SCT], dt.float32, tag="rec", name="rec")

                chunk_data = []
                for ci in range(SCT // CH):
                    pA = psA.tile([P, CH * P], dt.float32, tag="pA", name="pA")
                    for i in range(CH):
                        t = sc * SCT + ci * CH + i
                        lhs = xm[:, (ci * CH + i) * P:(ci * CH + i + 1) * P]
                        nc.tensor.matmul(out=pA[:, i * P:(i + 1) * P], lhsT=lhs,
                                         rhs=wcm[:], start=True, stop=True)
                        nc.tensor.matmul(out=pU[:, (ci * CH + i) * 4:(ci * CH + i + 1) * 4],
                                         lhsT=lhs, rhs=uuc[:], start=False, stop=True)
                    chunk_data.append(pA)

                # softmax over the whole superchunk
                nc.scalar.activation(qe[:], pU[:], mybir.ActivationFunctionType.Exp)
                nc.vector.tensor_reduce(
                    out=den[:], in_=APn(qe, [[4, SCT], [1, 4]]),
                    op=mybir.AluOpType.add, axis=mybir.AxisListType.X)
                nc.vector.reciprocal(out=rec[:], in_=den[:])
                nc.vector.tensor_tensor(
                    out=APn(qb, [[4, SCT], [1, 4]]),
                    in0=APn(qe, [[4, SCT], [1, 4]]),
                    in1=APn(rec, [[1, SCT], [0, 4]]),
                    op=mybir.AluOpType.mult)

                for ci in range(SCT // CH):
                    pA = chunk_data[ci]
                    stg = wp.tile([P, CH * P], dt.bfloat16, tag="stg", name="stg")
                    oh = wp.tile([P, CH * BLK], dt.bfloat16, tag="oh", name="oh")
                    # z~ = q * xjw  (plain layout: col i*128 + h*32 + ch)
                    nc.vector.tensor_tensor(
                        out=APn(stg, [[P, CH], [32, 4], [1, 32]]),
                        in0=APn(pA, [[P, CH], [32, 4], [1, 32]]),
                        in1=APn(qb, [[4, CH], [1, 4], [0, 32]], off=ci * CH * 4),
                        op=mybir.AluOpType.mult)
                    # onehot[e, i*64+dd] = (iota == dl)
                    nc.vector.tensor_tensor(
                        out=APn(oh, [[1, CH * BLK]]),
                        in0=APn(iota, [[1, CH * BLK]]),
                        in1=APn(dlb, [[1, CH], [0, BLK]], off=sc * SCT + ci * CH),
                        op=mybir.AluOpType.is_equal)
                    for i in range(CH):
                        t = sc * SCT + ci * CH + i
                        k = int(blk_of[t])
                        if k % 2 == 0 and t == t0[k]:
                            acc = psG.tile([P, P], dt.float32, tag="acc", name="acc")
                        half = (k % 2) * BLK
                        nc.tensor.matmul(
                            out=acc[half:half + BLK, :],
                            lhsT=oh[:, i * BLK:(i + 1) * BLK],
                            rhs=stg[:, i * P:(i + 1) * P],
                            start=(t == t0[k]), stop=(t == t0[k + 1] - 1))
                        if k % 2 == 1 and t == t0[k + 1] - 1:
                            m = k // 2
                            nc.scalar.copy(out=fin[:, m * P:(m + 1) * P], in_=acc[:])
                            if (m + 1) in FIN_BOUNDS:
                                emit_finale(FIN_BOUNDS[FIN_BOUNDS.index(m + 1) - 1]
                                            if FIN_BOUNDS.index(m + 1) > 0 else 0,
                                            m + 1)
    nc.compile()
    return nc


def kernel(x_v, edge_index_v, x_f, edge_index_f, Wv, Uv, cv, bv, Wf, Uf, cf, bf):
    _register_ntff_hook()
    import ml_dtypes
    from concourse import bass_utils

    x_v = np.asarray(x_v, np.float32)
    x_f = np.asarray(x_f, np.float32)
    cores = []
    for bi, (x, ei, W, U, c, b) in enumerate([
            (x_v, edge_index_v, Wv, Uv, cv, bv),
            (x_f, edge_index_f, Wf, Uf, cf, bf)]):
        ei = np.asarray(ei)
        s0, d0 = ei[0].astype(np.int64), ei[1].astype(np.int64)
        m = s0 != d0
        loops = np.arange(N, dtype=np.int64)
        src = np.concatenate([s0[m], loops])
        dst = np.concatenate([d0[m], loops])
        x16 = x.astype(ml_dtypes.bfloat16)
        for j in range(4):
            lo = j * NPC
            cores.append({
                "x16": x16, "W": np.asarray(W, np.float32),
                "U": np.asarray(U, np.float32), "c": np.asarray(c, np.float32),
                "b": np.asarray(b, np.float32), "lo": lo,
                "g": _prep_core(x16, src, dst, lo),
            })

    tn = np.stack([np.ceil(c["g"]["cnt"] / P).astype(np.int64) for c in cores])
    TPB = tn.max(axis=0)
    TPB = np.maximum(TPB, 1)
    NT = int(TPB.sum())
    pad = (-NT) % SCT
    TPB[NBLK - 1] += pad
    NT += pad
    base = np.concatenate([[0], np.cumsum(TPB)])[:-1]

    in_maps = []
    for c in cores:
        arrs = _build_core_arrays(None, c, TPB, base, NT)
        in_maps.append(arrs)

    nc = _build_program(TPB, NT)
    res = bass_utils.run_bass_kernel_spmd(
        nc, in_maps, core_ids=list(range(NCORES)),
        trace=bool(int(__import__("os").environ.get("KERNEL_TRACE", "0"))))
    kernel.last_result = res
    out_v = np.concatenate([res.results[j]["out"][:NPC] for j in range(4)])
    out_f = np.concatenate([res.results[4 + j]["out"][:NPC] for j in range(4)])
    return out_v, out_f


# revision 18
# speedup vs baseline: 1.2536x; 1.0734x over previous
"""FeaStConv dual-branch GNN message passing on 8 Trainium2 NeuronCores.

Sharding: branch v on cores 0-3, branch f on cores 4-7; each core owns a
12500-node destination range. Host reorders edges by destination block
(64 nodes), pre-gathers transposed source/dest features (bf16), device does
all float math: per-tile matmuls for x@W / (x_s-x_d)@U, softmax on-device,
one-hot scatter matmuls accumulating per-block in PSUM.
"""
import sys, types
import numpy as np

sys.path.insert(0, '/opt/trn_rl_repo')

N = 50000
IN_CH = 64
HEADS = 4
OUT_CH = 32
P = 128
NPC = 12500           # nodes per core
BLK = 64              # dst nodes per block
NBLK = 196            # blocks per core (196*64 = 12544)
NPAD = NBLK * BLK
CH = 12              # tiles per chunk
SCT = 24              # tiles per superchunk
NCORES = 8


def _register_ntff_hook():
    import antenv
    if "antenv.axon_hooks" in sys.modules:
        return
    mod = types.ModuleType("antenv.axon_hooks")
    _h = [None]
    mod.set_axon_ntff_profile_hook = lambda h: _h.__setitem__(0, h)
    mod.get_axon_ntff_profile_hook = lambda: _h[0]
    sys.modules["antenv.axon_hooks"] = mod
    antenv.axon_hooks = mod
    if "/root/.axon_site" not in sys.path:
        sys.path.insert(0, "/root/.axon_site")
    try:
        from trn_agent_boot.trn_boot import _ntff_profile_via_ctypes
        mod.set_axon_ntff_profile_hook(_ntff_profile_via_ctypes('/opt/axon/libaxon_pjrt.so'))
    except Exception:
        pass


def _prep_core(x16, src, dst, lo):
    """Per-core edge layout. Returns dict with per-block counts and sorted
    (global-src, global-dst, local-slot) arrays."""
    sel = (dst >= lo) & (dst < lo + NPC)
    s = src[sel]
    d = (dst[sel] - lo).astype(np.int64)
    order = np.argsort(d, kind='stable')
    s = s[order]
    d = d[order]
    blk = d >> 6
    cnt = np.bincount(blk, minlength=NBLK).astype(np.int64)
    deg = np.bincount(d, minlength=NPAD).astype(np.float32)
    return {"s": s, "d": d, "cnt": cnt, "deg": deg}


def _build_core_arrays(ml, core, TPB, base, NT):
    import ml_dtypes
    x16, W, U, c, b = core["x16"], core["W"], core["U"], core["c"], core["b"]
    s, d, cnt = core["g"]["s"], core["g"]["d"], core["g"]["cnt"]
    E_pad = NT * P
    srcg = np.zeros(E_pad, np.int64)
    dstg = np.zeros(E_pad, np.int64)
    dl = np.full(E_pad, -1.0, np.float32)
    # place each block's edges at its tile base
    cstart = np.concatenate([[0], np.cumsum(cnt)])
    for k in range(NBLK):
        n_k = int(cnt[k])
        if n_k == 0:
            continue
        p0 = base[k] * P
        srcg[p0:p0 + n_k] = s[cstart[k]:cstart[k] + n_k]
        dstg[p0:p0 + n_k] = d[cstart[k]:cstart[k] + n_k] + core["lo"]
        dl[p0:p0 + n_k] = (d[cstart[k]:cstart[k] + n_k] - BLK * k).astype(np.float32)
    xsd = np.empty((P, E_pad), ml_dtypes.bfloat16)
    xsd[:IN_CH, :] = x16[srcg].T
    xsd[IN_CH:, :] = x16[dstg].T
    dl16 = np.ascontiguousarray(dl.reshape(NT, P).T).astype(ml_dtypes.bfloat16)
    # plain layout [h*32+ch]; bottom 64 rows zero (x_dst doesn't enter xjw)
    Wcm = np.zeros((P, P), np.float32)
    Wcm[:IN_CH] = W
    UUc = np.concatenate([U, -U], axis=0)  # [128, 4]
    degp = np.ascontiguousarray(core["g"]["deg"].reshape(NBLK // 2, P).T)  # [128, 98]
    return {
        "xsd": xsd,
        "dl": dl16,
        "wcm": Wcm.astype(ml_dtypes.bfloat16),
        "uuc": UUc.astype(ml_dtypes.bfloat16),
        "crep": np.tile(c[None, :], (P, 1)).astype(np.float32),
        "cvec": np.tile(c, SCT)[None, :].astype(ml_dtypes.bfloat16),
        "brep": np.tile(b[None, :], (P, 1)).astype(np.float32),
        "degp": degp.astype(np.float32),
        "iota": np.tile(np.arange(BLK, dtype=np.float32)[None, :], (P, CH)).astype(ml_dtypes.bfloat16),
    }


def _build_program(TPB, NT):
    import concourse.bass as bass
    import concourse.mybir as mybir
    import concourse.bacc as bacc
    from concourse.tile import TileContext

    dt = mybir.dt
    NSC = NT // SCT
    NCH = NT // CH
    # block index / first / last flags per tile
    blk_of = np.repeat(np.arange(NBLK), TPB)
    t0 = np.concatenate([[0], np.cumsum(TPB)])

    nc = bacc.Bacc("TRN2", target_bir_lowering=False, debug=False, num_devices=NCORES)
    xsd_d = nc.dram_tensor("xsd", [P, NT * P], dt.bfloat16, kind="ExternalInput").ap()
    dl_d = nc.dram_tensor("dl", [P, NT], dt.bfloat16, kind="ExternalInput").ap()
    wcm_d = nc.dram_tensor("wcm", [P, P], dt.bfloat16, kind="ExternalInput").ap()
    uuc_d = nc.dram_tensor("uuc", [P, 4], dt.bfloat16, kind="ExternalInput").ap()
    crep_d = nc.dram_tensor("crep", [P, 4], dt.float32, kind="ExternalInput").ap()
    cvec_d = nc.dram_tensor("cvec", [1, SCT * 4], dt.bfloat16, kind="ExternalInput").ap()
    brep_d = nc.dram_tensor("brep", [P, OUT_CH], dt.float32, kind="ExternalInput").ap()
    degp_d = nc.dram_tensor("degp", [P, NBLK // 2], dt.float32, kind="ExternalInput").ap()
    iota_d = nc.dram_tensor("iota", [P, BLK * CH], dt.bfloat16, kind="ExternalInput").ap()
    out_d = nc.dram_tensor("out", [NPAD, OUT_CH], dt.float32, kind="ExternalOutput").ap()

    def APn(t, dims, off=0):
        a = t[:]
        return bass.AP(a.tensor, a.offset + off, [a.ap[0]] + dims)

    with TileContext(nc) as tc:
        with tc.tile_pool(name="const", bufs=1) as cp, \
             tc.tile_pool(name="mega", bufs=3) as mp, \
             tc.tile_pool(name="work", bufs=6) as wp, \
             tc.tile_pool(name="qp", bufs=2) as qp, \
             tc.tile_pool(name="fin", bufs=2) as fp, \
             tc.tile_pool(name="finacc", bufs=1) as fap, \
             tc.tile_pool(name="psA", bufs=2, space="PSUM") as psA, \
             tc.tile_pool(name="psU", bufs=1, space="PSUM") as psU, \
             tc.tile_pool(name="psG", bufs=1, space="PSUM") as psG:

            wcm = cp.tile([P, P], dt.bfloat16)
            uuc = cp.tile([P, 4], dt.bfloat16)
            crep = cp.tile([P, 4], dt.float32)
            brep = cp.tile([P, OUT_CH], dt.float32)
            degp = cp.tile([P, NBLK // 2], dt.float32)
            iota = cp.tile([P, BLK * CH], dt.bfloat16)
            dlb = cp.tile([P, NT], dt.bfloat16)
            cvec = cp.tile([1, SCT * 4], dt.bfloat16)
            ones1 = cp.tile([1, P], dt.bfloat16)
            expc = cp.tile([P, 4], dt.float32)
            nc.sync.dma_start(out=wcm[:], in_=wcm_d[:])
            nc.sync.dma_start(out=uuc[:], in_=uuc_d[:])
            nc.sync.dma_start(out=crep[:], in_=crep_d[:])
            nc.sync.dma_start(out=brep[:], in_=brep_d[:])
            nc.sync.dma_start(out=degp[:], in_=degp_d[:])
            nc.sync.dma_start(out=iota[:], in_=iota_d[:])
            nc.sync.dma_start(out=dlb[:], in_=dl_d[:])
            nc.sync.dma_start(out=cvec[:], in_=cvec_d[:])
            nc.vector.memset(ones1[:], 1.0)
            nc.scalar.activation(expc[:], crep[:], mybir.ActivationFunctionType.Exp)

            fin = fap.tile([P, (NBLK // 2) * P], dt.float32)

            NH = NBLK // 2
            FIN_BOUNDS = [25, 50, 75, NH]

            def emit_finale(g0, g1):
                ng = g1 - g0
                hs = fp.tile([P, ng * OUT_CH], dt.float32, tag="hs", name="hs")
                nc.vector.tensor_reduce(
                    out=APn(hs, [[32, ng], [1, 32]]),
                    in_=APn(fin, [[P, ng], [1, 32], [32, 4]], off=g0 * P),
                    op=mybir.AluOpType.add, axis=mybir.AxisListType.X)
                dmx = fp.tile([P, ng], dt.float32, tag="dmx", name="dmx")
                nc.vector.tensor_scalar(out=dmx[:], in0=degp[:, g0:g1],
                                        scalar1=1.0, scalar2=None,
                                        op0=mybir.AluOpType.max)
                drc = fp.tile([P, ng], dt.float32, tag="drc", name="drc")
                nc.vector.reciprocal(out=drc[:], in_=dmx[:])
                o1 = fp.tile([P, ng * OUT_CH], dt.float32, tag="o1", name="o1")
                nc.vector.tensor_tensor(
                    out=APn(o1, [[32, ng], [1, 32]]),
                    in0=APn(hs, [[32, ng], [1, 32]]),
                    in1=APn(drc, [[1, ng], [0, 32]]),
                    op=mybir.AluOpType.mult)
                nc.vector.tensor_tensor(
                    out=APn(o1, [[32, ng], [1, 32]]),
                    in0=APn(o1, [[32, ng], [1, 32]]),
                    in1=APn(brep, [[0, ng], [1, 32]]),
                    op=mybir.AluOpType.add)
                o2 = fp.tile([P, ng * OUT_CH], dt.float32, tag="o2", name="o2")
                nc.vector.tensor_scalar(out=o2[:], in0=o1[:], scalar1=0.2,
                                        scalar2=None, op0=mybir.AluOpType.mult)
                nc.vector.tensor_tensor(out=o1[:], in0=o1[:], in1=o2[:],
                                        op=mybir.AluOpType.max)
                out_ap = bass.AP(out_d.tensor, out_d.offset + g0 * P * OUT_CH,
                                 [[OUT_CH, P], [P * OUT_CH, ng], [1, OUT_CH]])
                nc.sync.dma_start(out=out_ap, in_=APn(o1, [[32, ng], [1, 32]]))

            acc = None
            for sc in range(NSC):
                xm = mp.tile([P, SCT * P], dt.bfloat16, tag="xm", name="xm")
                nc.sync.dma_start(out=xm[:], in_=xsd_d[:, sc * SCT * P:(sc + 1) * SCT * P])
                pU = psU.tile([P, SCT * 4], dt.float32, tag="pU", name="pU")
                nc.tensor.matmul(out=pU[:], lhsT=ones1[:], rhs=cvec[:],
                                 start=True, stop=False)
                qe = qp.tile([P, SCT * 4], dt.float32, tag="qe", name="qe")
                qb = qp.tile([P, SCT * 4], dt.float32, tag="qb", name="qb")
                den = qp.tile([P, SCT], dt.float32, tag="den", name="den")
                rec = qp.tile([P, SCT], dt.float32, tag="rec", name="rec")

                chunk_data = []
                for ci in range(SCT // CH):
                    pA = psA.tile([P, CH * P], dt.float32, tag="pA", name="pA")
                    for i in range(CH):
                        t = sc * SCT + ci * CH + i
                        lhs = xm[:, (ci * CH + i) * P:(ci * CH + i + 1) * P]
                        nc.tensor.matmul(out=pA[:, i * P:(i + 1) * P], lhsT=lhs,
                                         rhs=wcm[:], start=True, stop=True)
                        nc.tensor.matmul(out=pU[:, (ci * CH + i) * 4:(ci * CH + i + 1) * 4],
                                         lhsT=lhs, rhs=uuc[:], start=False, stop=True)
                    chunk_data.append(pA)

                # softmax over the whole superchunk
                nc.scalar.activation(qe[:], pU[:], mybir.ActivationFunctionType.Exp)
                nc.vector.tensor_reduce(
                    out=den[:], in_=APn(qe, [[4, SCT], [1, 4]]),
                    op=mybir.AluOpType.add, axis=mybir.AxisListType.X)
                nc.vector.reciprocal(out=rec[:], in_=den[:])
                nc.vector.tensor_tensor(
                    out=APn(qb, [[4, SCT], [1, 4]]),
                    in0=APn(qe, [[4, SCT], [1, 4]]),
                    in1=APn(rec, [[1, SCT], [0, 4]]),
                    op=mybir.AluOpType.mult)

                for ci in range(SCT // CH):
                    pA = chunk_data[ci]
                    stg = wp.tile([P, CH * P], dt.bfloat16, tag="stg", name="stg")
                    oh = wp.tile([P, CH * BLK], dt.bfloat16, tag="oh", name="oh")
                    # z~ = q * xjw  (plain layout: col i*128 + h*32 + ch)
                    nc.vector.tensor_tensor(
                        out=APn(stg, [[P, CH], [32, 4], [1, 32]]),
                        in0=APn(pA, [[P, CH], [32, 4], [1, 32]]),
                        in1=APn(qb, [[4, CH], [1, 4], [0, 32]], off=ci * CH * 4),
                        op=mybir.AluOpType.mult)
                    # onehot[e, i*64+dd] = (iota == dl)
                    nc.vector.tensor_tensor(
                        out=APn(oh, [[1, CH * BLK]]),
                        in0=APn(iota, [[1, CH * BLK]]),
                        in1=APn(dlb, [[1, CH], [0, BLK]], off=sc * SCT + ci * CH),
                        op=mybir.AluOpType.is_equal)
                    for i in range(CH):
                        t = sc * SCT + ci * CH + i
                        k = int(blk_of[t])
                        if k % 2 == 0 and t == t0[k]:
                            acc = psG.tile([P, P], dt.float32, tag="acc", name="acc")
                        half = (k % 2) * BLK
                        nc.tensor.matmul(
                            out=acc[half:half + BLK, :],
                            lhsT=oh[:, i * BLK:(i + 1) * BLK],
                            rhs=stg[:, i * P:(i + 1) * P],
                            start=(t == t0[k]), stop=(t == t0[k + 1] - 1))
                        if k % 2 == 1 and t == t0[k + 1] - 1:
                            m = k // 2
                            nc.scalar.copy(out=fin[:, m * P:(m + 1) * P], in_=acc[:])
                            if (m + 1) in FIN_BOUNDS:
                                emit_finale(FIN_BOUNDS[FIN_BOUNDS.index(m + 1) - 1]
                                            if FIN_BOUNDS.index(m + 1) > 0 else 0,
                                            m + 1)
    nc.compile()
    return nc


def kernel(x_v, edge_index_v, x_f, edge_index_f, Wv, Uv, cv, bv, Wf, Uf, cf, bf):
    _register_ntff_hook()
    import ml_dtypes
    from concourse import bass_utils

    x_v = np.asarray(x_v, np.float32)
    x_f = np.asarray(x_f, np.float32)
    cores = []
    for bi, (x, ei, W, U, c, b) in enumerate([
            (x_v, edge_index_v, Wv, Uv, cv, bv),
            (x_f, edge_index_f, Wf, Uf, cf, bf)]):
        ei = np.asarray(ei)
        s0, d0 = ei[0].astype(np.int64), ei[1].astype(np.int64)
        m = s0 != d0
        loops = np.arange(N, dtype=np.int64)
        src = np.concatenate([s0[m], loops])
        dst = np.concatenate([d0[m], loops])
        x16 = x.astype(ml_dtypes.bfloat16)
        for j in range(4):
            lo = j * NPC
            cores.append({
                "x16": x16, "W": np.asarray(W, np.float32),
                "U": np.asarray(U, np.float32), "c": np.asarray(c, np.float32),
                "b": np.asarray(b, np.float32), "lo": lo,
                "g": _prep_core(x16, src, dst, lo),
            })

    tn = np.stack([np.ceil(c["g"]["cnt"] / P).astype(np.int64) for c in cores])
    TPB = tn.max(axis=0)
    TPB = np.maximum(TPB, 1)
    NT = int(TPB.sum())
    pad = (-NT) % SCT
    TPB[NBLK - 1] += pad
    NT += pad
    base = np.concatenate([[0], np.cumsum(TPB)])[:-1]

    in_maps = []
    for c in cores:
        arrs = _build_core_arrays(None, c, TPB, base, NT)
        in_maps.append(arrs)

    nc = _build_program(TPB, NT)
    res = bass_utils.run_bass_kernel_spmd(
        nc, in_maps, core_ids=list(range(NCORES)),
        trace=bool(int(__import__("os").environ.get("KERNEL_TRACE", "0"))))
    kernel.last_result = res
    out_v = np.concatenate([res.results[j]["out"][:NPC] for j in range(4)])
    out_f = np.concatenate([res.results[4 + j]["out"][:NPC] for j in range(4)])
    return out_v, out_f


# revision 19
# speedup vs baseline: 1.2622x; 1.0068x over previous
"""FeaStConv dual-branch GNN message passing on 8 Trainium2 NeuronCores.

Sharding: branch v on cores 0-3, branch f on cores 4-7; each core owns a
12500-node destination range. Host reorders edges by destination block
(64 nodes), pre-gathers transposed source/dest features (bf16), device does
all float math: per-tile matmuls for x@W / (x_s-x_d)@U, softmax on-device,
one-hot scatter matmuls accumulating per-block in PSUM.
"""
import sys, types
import numpy as np

sys.path.insert(0, '/opt/trn_rl_repo')

N = 50000
IN_CH = 64
HEADS = 4
OUT_CH = 32
P = 128
NPC = 12500           # nodes per core
BLK = 64              # dst nodes per block
NBLK = 196            # blocks per core (196*64 = 12544)
NPAD = NBLK * BLK
CH = 12              # tiles per chunk
SCT = 36              # tiles per superchunk
NCORES = 8


def _register_ntff_hook():
    import antenv
    if "antenv.axon_hooks" in sys.modules:
        return
    mod = types.ModuleType("antenv.axon_hooks")
    _h = [None]
    mod.set_axon_ntff_profile_hook = lambda h: _h.__setitem__(0, h)
    mod.get_axon_ntff_profile_hook = lambda: _h[0]
    sys.modules["antenv.axon_hooks"] = mod
    antenv.axon_hooks = mod
    if "/root/.axon_site" not in sys.path:
        sys.path.insert(0, "/root/.axon_site")
    try:
        from trn_agent_boot.trn_boot import _ntff_profile_via_ctypes
        mod.set_axon_ntff_profile_hook(_ntff_profile_via_ctypes('/opt/axon/libaxon_pjrt.so'))
    except Exception:
        pass


def _prep_core(x16, src, dst, lo):
    """Per-core edge layout. Returns dict with per-block counts and sorted
    (global-src, global-dst, local-slot) arrays."""
    sel = (dst >= lo) & (dst < lo + NPC)
    s = src[sel]
    d = (dst[sel] - lo).astype(np.int64)
    order = np.argsort(d, kind='stable')
    s = s[order]
    d = d[order]
    blk = d >> 6
    cnt = np.bincount(blk, minlength=NBLK).astype(np.int64)
    deg = np.bincount(d, minlength=NPAD).astype(np.float32)
    return {"s": s, "d": d, "cnt": cnt, "deg": deg}


def _build_core_arrays(ml, core, TPB, base, NT):
    import ml_dtypes
    x16, W, U, c, b = core["x16"], core["W"], core["U"], core["c"], core["b"]
    s, d, cnt = core["g"]["s"], core["g"]["d"], core["g"]["cnt"]
    E_pad = NT * P
    srcg = np.zeros(E_pad, np.int64)
    dstg = np.zeros(E_pad, np.int64)
    dl = np.full(E_pad, -1.0, np.float32)
    # place each block's edges at its tile base
    cstart = np.concatenate([[0], np.cumsum(cnt)])
    for k in range(NBLK):
        n_k = int(cnt[k])
        if n_k == 0:
            continue
        p0 = base[k] * P
        srcg[p0:p0 + n_k] = s[cstart[k]:cstart[k] + n_k]
        dstg[p0:p0 + n_k] = d[cstart[k]:cstart[k] + n_k] + core["lo"]
        dl[p0:p0 + n_k] = (d[cstart[k]:cstart[k] + n_k] - BLK * k).astype(np.float32)
    xsd = np.empty((P, E_pad), ml_dtypes.bfloat16)
    xsd[:IN_CH, :] = x16[srcg].T
    xsd[IN_CH:, :] = x16[dstg].T
    dl16 = np.ascontiguousarray(dl.reshape(NT, P).T).astype(ml_dtypes.bfloat16)
    # plain layout [h*32+ch]; bottom 64 rows zero (x_dst doesn't enter xjw)
    Wcm = np.zeros((P, P), np.float32)
    Wcm[:IN_CH] = W
    UUc = np.concatenate([U, -U], axis=0)  # [128, 4]
    degp = np.ascontiguousarray(core["g"]["deg"].reshape(NBLK // 2, P).T)  # [128, 98]
    return {
        "xsd": xsd,
        "dl": dl16,
        "wcm": Wcm.astype(ml_dtypes.bfloat16),
        "uuc": UUc.astype(ml_dtypes.bfloat16),
        "crep": np.tile(c[None, :], (P, 1)).astype(np.float32),
        "cvec": np.tile(c, SCT)[None, :].astype(ml_dtypes.bfloat16),
        "brep": np.tile(b[None, :], (P, 1)).astype(np.float32),
        "degp": degp.astype(np.float32),
        "iota": np.tile(np.arange(BLK, dtype=np.float32)[None, :], (P, CH)).astype(ml_dtypes.bfloat16),
    }


def _build_program(TPB, NT):
    import concourse.bass as bass
    import concourse.mybir as mybir
    import concourse.bacc as bacc
    from concourse.tile import TileContext

    dt = mybir.dt
    NSC = NT // SCT
    NCH = NT // CH
    # block index / first / last flags per tile
    blk_of = np.repeat(np.arange(NBLK), TPB)
    t0 = np.concatenate([[0], np.cumsum(TPB)])

    nc = bacc.Bacc("TRN2", target_bir_lowering=False, debug=False, num_devices=NCORES)
    xsd_d = nc.dram_tensor("xsd", [P, NT * P], dt.bfloat16, kind="ExternalInput").ap()
    dl_d = nc.dram_tensor("dl", [P, NT], dt.bfloat16, kind="ExternalInput").ap()
    wcm_d = nc.dram_tensor("wcm", [P, P], dt.bfloat16, kind="ExternalInput").ap()
    uuc_d = nc.dram_tensor("uuc", [P, 4], dt.bfloat16, kind="ExternalInput").ap()
    crep_d = nc.dram_tensor("crep", [P, 4], dt.float32, kind="ExternalInput").ap()
    cvec_d = nc.dram_tensor("cvec", [1, SCT * 4], dt.bfloat16, kind="ExternalInput").ap()
    brep_d = nc.dram_tensor("brep", [P, OUT_CH], dt.float32, kind="ExternalInput").ap()
    degp_d = nc.dram_tensor("degp", [P, NBLK // 2], dt.float32, kind="ExternalInput").ap()
    iota_d = nc.dram_tensor("iota", [P, BLK * CH], dt.bfloat16, kind="ExternalInput").ap()
    out_d = nc.dram_tensor("out", [NPAD, OUT_CH], dt.float32, kind="ExternalOutput").ap()

    def APn(t, dims, off=0):
        a = t[:]
        return bass.AP(a.tensor, a.offset + off, [a.ap[0]] + dims)

    with TileContext(nc) as tc:
        with tc.tile_pool(name="const", bufs=1) as cp, \
             tc.tile_pool(name="mega", bufs=3) as mp, \
             tc.tile_pool(name="work", bufs=6) as wp, \
             tc.tile_pool(name="qp", bufs=2) as qp, \
             tc.tile_pool(name="fin", bufs=2) as fp, \
             tc.tile_pool(name="finacc", bufs=1) as fap, \
             tc.tile_pool(name="psA", bufs=2, space="PSUM") as psA, \
             tc.tile_pool(name="psU", bufs=1, space="PSUM") as psU, \
             tc.tile_pool(name="psG", bufs=1, space="PSUM") as psG:

            wcm = cp.tile([P, P], dt.bfloat16)
            uuc = cp.tile([P, 4], dt.bfloat16)
            crep = cp.tile([P, 4], dt.float32)
            brep = cp.tile([P, OUT_CH], dt.float32)
            degp = cp.tile([P, NBLK // 2], dt.float32)
            iota = cp.tile([P, BLK * CH], dt.bfloat16)
            dlb = cp.tile([P, NT], dt.bfloat16)
            cvec = cp.tile([1, SCT * 4], dt.bfloat16)
            ones1 = cp.tile([1, P], dt.bfloat16)
            expc = cp.tile([P, 4], dt.float32)
            nc.sync.dma_start(out=wcm[:], in_=wcm_d[:])
            nc.sync.dma_start(out=uuc[:], in_=uuc_d[:])
            nc.sync.dma_start(out=crep[:], in_=crep_d[:])
            nc.sync.dma_start(out=brep[:], in_=brep_d[:])
            nc.sync.dma_start(out=degp[:], in_=degp_d[:])
            nc.sync.dma_start(out=iota[:], in_=iota_d[:])
            nc.sync.dma_start(out=dlb[:], in_=dl_d[:])
            nc.sync.dma_start(out=cvec[:], in_=cvec_d[:])
            nc.vector.memset(ones1[:], 1.0)
            nc.scalar.activation(expc[:], crep[:], mybir.ActivationFunctionType.Exp)

            fin = fap.tile([P, (NBLK // 2) * P], dt.float32)

            NH = NBLK // 2
            FIN_BOUNDS = [25, 50, 75, NH]

            def emit_finale(g0, g1):
                ng = g1 - g0
                hs = fp.tile([P, ng * OUT_CH], dt.float32, tag="hs", name="hs")
                nc.vector.tensor_reduce(
                    out=APn(hs, [[32, ng], [1, 32]]),
                    in_=APn(fin, [[P, ng], [1, 32], [32, 4]], off=g0 * P),
                    op=mybir.AluOpType.add, axis=mybir.AxisListType.X)
                dmx = fp.tile([P, ng], dt.float32, tag="dmx", name="dmx")
                nc.vector.tensor_scalar(out=dmx[:], in0=degp[:, g0:g1],
                                        scalar1=1.0, scalar2=None,
                                        op0=mybir.AluOpType.max)
                drc = fp.tile([P, ng], dt.float32, tag="drc", name="drc")
                nc.vector.reciprocal(out=drc[:], in_=dmx[:])
                o1 = fp.tile([P, ng * OUT_CH], dt.float32, tag="o1", name="o1")
                nc.vector.tensor_tensor(
                    out=APn(o1, [[32, ng], [1, 32]]),
                    in0=APn(hs, [[32, ng], [1, 32]]),
                    in1=APn(drc, [[1, ng], [0, 32]]),
                    op=mybir.AluOpType.mult)
                nc.vector.tensor_tensor(
                    out=APn(o1, [[32, ng], [1, 32]]),
                    in0=APn(o1, [[32, ng], [1, 32]]),
                    in1=APn(brep, [[0, ng], [1, 32]]),
                    op=mybir.AluOpType.add)
                o2 = fp.tile([P, ng * OUT_CH], dt.float32, tag="o2", name="o2")
                nc.vector.tensor_scalar(out=o2[:], in0=o1[:], scalar1=0.2,
                                        scalar2=None, op0=mybir.AluOpType.mult)
                nc.vector.tensor_tensor(out=o1[:], in0=o1[:], in1=o2[:],
                                        op=mybir.AluOpType.max)
                out_ap = bass.AP(out_d.tensor, out_d.offset + g0 * P * OUT_CH,
                                 [[OUT_CH, P], [P * OUT_CH, ng], [1, OUT_CH]])
                nc.sync.dma_start(out=out_ap, in_=APn(o1, [[32, ng], [1, 32]]))

            acc = None
            for sc in range(NSC):
                xm = mp.tile([P, SCT * P], dt.bfloat16, tag="xm", name="xm")
                nc.sync.dma_start(out=xm[:], in_=xsd_d[:, sc * SCT * P:(sc + 1) * SCT * P])
                pU = psU.tile([P, SCT * 4], dt.float32, tag="pU", name="pU")
                nc.tensor.matmul(out=pU[:], lhsT=ones1[:], rhs=cvec[:],
                                 start=True, stop=False)
                qe = qp.tile([P, SCT * 4], dt.float32, tag="qe", name="qe")
                qb = qp.tile([P, SCT * 4], dt.float32, tag="qb", name="qb")
                den = qp.tile([P, SCT], dt.float32, tag="den", name="den")
                rec = qp.tile([P, SCT], dt.float32, tag="rec", name="rec")

                chunk_data = []
                for ci in range(SCT // CH):
                    pA = psA.tile([P, CH * P], dt.float32, tag="pA", name="pA")
                    for i in range(CH):
                        t = sc * SCT + ci * CH + i
                        lhs = xm[:, (ci * CH + i) * P:(ci * CH + i + 1) * P]
                        nc.tensor.matmul(out=pA[:, i * P:(i + 1) * P], lhsT=lhs,
                                         rhs=wcm[:], start=True, stop=True)
                        nc.tensor.matmul(out=pU[:, (ci * CH + i) * 4:(ci * CH + i + 1) * 4],
                                         lhsT=lhs, rhs=uuc[:], start=False, stop=True)
                    chunk_data.append(pA)

                # softmax over the whole superchunk
                nc.scalar.activation(qe[:], pU[:], mybir.ActivationFunctionType.Exp)
                nc.vector.tensor_reduce(
                    out=den[:], in_=APn(qe, [[4, SCT], [1, 4]]),
                    op=mybir.AluOpType.add, axis=mybir.AxisListType.X)
                nc.vector.reciprocal(out=rec[:], in_=den[:])
                nc.vector.tensor_tensor(
                    out=APn(qb, [[4, SCT], [1, 4]]),
                    in0=APn(qe, [[4, SCT], [1, 4]]),
                    in1=APn(rec, [[1, SCT], [0, 4]]),
                    op=mybir.AluOpType.mult)

                for ci in range(SCT // CH):
                    pA = chunk_data[ci]
                    stg = wp.tile([P, CH * P], dt.bfloat16, tag="stg", name="stg")
                    oh = wp.tile([P, CH * BLK], dt.bfloat16, tag="oh", name="oh")
                    # z~ = q * xjw  (plain layout: col i*128 + h*32 + ch)
                    nc.vector.tensor_tensor(
                        out=APn(stg, [[P, CH], [32, 4], [1, 32]]),
                        in0=APn(pA, [[P, CH], [32, 4], [1, 32]]),
                        in1=APn(qb, [[4, CH], [1, 4], [0, 32]], off=ci * CH * 4),
                        op=mybir.AluOpType.mult)
                    # onehot[e, i*64+dd] = (iota == dl)
                    nc.vector.tensor_tensor(
                        out=APn(oh, [[1, CH * BLK]]),
                        in0=APn(iota, [[1, CH * BLK]]),
                        in1=APn(dlb, [[1, CH], [0, BLK]], off=sc * SCT + ci * CH),
                        op=mybir.AluOpType.is_equal)
                    for i in range(CH):
                        t = sc * SCT + ci * CH + i
                        k = int(blk_of[t])
                        if k % 2 == 0 and t == t0[k]:
                            acc = psG.tile([P, P], dt.float32, tag="acc", name="acc")
                        half = (k % 2) * BLK
                        nc.tensor.matmul(
                            out=acc[half:half + BLK, :],
                            lhsT=oh[:, i * BLK:(i + 1) * BLK],
                            rhs=stg[:, i * P:(i + 1) * P],
                            start=(t == t0[k]), stop=(t == t0[k + 1] - 1))
                        if k % 2 == 1 and t == t0[k + 1] - 1:
                            m = k // 2
                            nc.scalar.copy(out=fin[:, m * P:(m + 1) * P], in_=acc[:])
                            if (m + 1) in FIN_BOUNDS:
                                emit_finale(FIN_BOUNDS[FIN_BOUNDS.index(m + 1) - 1]
                                            if FIN_BOUNDS.index(m + 1) > 0 else 0,
                                            m + 1)
    nc.compile()
    return nc


def kernel(x_v, edge_index_v, x_f, edge_index_f, Wv, Uv, cv, bv, Wf, Uf, cf, bf):
    _register_ntff_hook()
    import ml_dtypes
    from concourse import bass_utils

    x_v = np.asarray(x_v, np.float32)
    x_f = np.asarray(x_f, np.float32)
    cores = []
    for bi, (x, ei, W, U, c, b) in enumerate([
            (x_v, edge_index_v, Wv, Uv, cv, bv),
            (x_f, edge_index_f, Wf, Uf, cf, bf)]):
        ei = np.asarray(ei)
        s0, d0 = ei[0].astype(np.int64), ei[1].astype(np.int64)
        m = s0 != d0
        loops = np.arange(N, dtype=np.int64)
        src = np.concatenate([s0[m], loops])
        dst = np.concatenate([d0[m], loops])
        x16 = x.astype(ml_dtypes.bfloat16)
        for j in range(4):
            lo = j * NPC
            cores.append({
                "x16": x16, "W": np.asarray(W, np.float32),
                "U": np.asarray(U, np.float32), "c": np.asarray(c, np.float32),
                "b": np.asarray(b, np.float32), "lo": lo,
                "g": _prep_core(x16, src, dst, lo),
            })

    tn = np.stack([np.ceil(c["g"]["cnt"] / P).astype(np.int64) for c in cores])
    TPB = tn.max(axis=0)
    TPB = np.maximum(TPB, 1)
    NT = int(TPB.sum())
    pad = (-NT) % SCT
    TPB[NBLK - 1] += pad
    NT += pad
    base = np.concatenate([[0], np.cumsum(TPB)])[:-1]

    in_maps = []
    for c in cores:
        arrs = _build_core_arrays(None, c, TPB, base, NT)
        in_maps.append(arrs)

    nc = _build_program(TPB, NT)
    res = bass_utils.run_bass_kernel_spmd(
        nc, in_maps, core_ids=list(range(NCORES)),
        trace=bool(int(__import__("os").environ.get("KERNEL_TRACE", "0"))))
    kernel.last_result = res
    out_v = np.concatenate([res.results[j]["out"][:NPC] for j in range(4)])
    out_f = np.concatenate([res.results[4 + j]["out"][:NPC] for j in range(4)])
    return out_v, out_f
